# revision 26
# baseline (speedup 1.0000x reference)
"""Trainium2 Bass kernel for ChebyshevLayer — hybrid bf16 / fp8-DoubleRow.

Math:
    t = tanh(x)                                   [B, IN]
    out = sum_n T_n(t) @ coeffs[:, :, n] + x @ base_weight
T_0 == 1 collapses to a bias row bias[o] = sum_i coeffs[i, o, 0].  The rest
is one contraction over 11 K-planes of 1024 rows: [T1(=t), x, T2..T10]
against [c1, base_weight, c2..c10].

Precision plan: the rel-err gate is 2e-2; bf16 everywhere measures ~2.9e-3.
Planes in FP8_PLANES run as e4m3 x e4m3 matmuls in MatmulPerfMode.DoubleRow
(0.5 PE cycles/row instead of 1.0 — two K-chunks per instruction), spending
~8.7e-3 of quadrature error per plane to cut that plane's PE time in half.
Scales: A-side x128, W-side x512 (both exact powers of two), PSUM drained
with a x2^-16 fused multiply.

Host-side prep (inside kernel(), numpy): x is pre-transposed per 128-row
block into [NBLK, 128, NCH, 128] ([i%128, chunk, batch] per block) so the
PE transposes of the old design disappear; W planes are pre-cast (bf16) or
pre-quantized (fp8, pair-interleaved [128, NPAIR, 2, OC] for DoubleRow's
two-slab operand layout) and DMA'd as whole planes (1-2 large DMAs each —
the SP sequencer costs 565ns per DMA issue, so few/large transfers keep
the stream at full 360 GB/s).

Startup: W streams plane-major, fp8 planes first (smallest, and T1..T3 are
ready earliest in the basis chain).  The first SJ blocks issue per-plane
singleton PSUM groups that follow the stream, accumulated in SBUF by DVE;
bias comes from an 8-matmul ones-reduction over the streamed c0 plane.
Steady state: per block one 56-matmul bf16 group [128, OC] plus two
24-matmul fp8 DoubleRow groups [64, OC] (batch halves; each group spans
both 256-col output halves of one PSUM bank), drained by DVE with fused
scale+combine+bias adds.

Sharding over 8 cores: batch x4, out-features x2.
Per core: x [2048, 1024], coeffs [1024, 512, 11], bw [1024, 512]
          -> out [2048, 512].
"""

import numpy as np
import ml_dtypes

import concourse.bass as bass
import concourse.mybir as mybir
import concourse.tile as tile
from concourse import bacc
from concourse.bass_utils import run_bass_kernel_spmd

F32 = mybir.dt.float32
BF16 = mybir.dt.bfloat16
F8 = mybir.dt.float8e4
F16 = mybir.dt.float16
AF = mybir.ActivationFunctionType
OP = mybir.AluOpType
PM = mybir.MatmulPerfMode

B, IN, OUT = 8192, 1024, 1024
DEG = 10
MB, MO = 4, 2                    # batch shards x out-feature shards
BC, OC = B // MB, OUT // MO      # per-core 2048 batch rows, 512 out cols
NBLK = BC // 128                 # 16 batch blocks per core
NCH = IN // 128                  # 8 contraction chunks
NPAIR = NCH // 2                 # 4 chunk pairs (DoubleRow does 2 at once)
NKB = DEG + 1                    # 11 K-planes: 0=T1, 1=x, 2..10=T2..T10
HALF = OC // 2

FP8_PLANES = (0, 2, 3, 4)        # planes run as fp8 DoubleRow
BF16_PLANES = tuple(bi for bi in range(NKB) if bi not in FP8_PLANES)
SA, SW = 128.0, 512.0            # A-side / W-side fp8 scales (powers of 2)
SINV = 1.0 / (SA * SW)
SQRT2 = float(np.sqrt(2.0))
SJ = 2                           # startup blocks with per-plane groups

_CACHE = {}
LAST_RESULTS = None


def _build_nc():
    nc = bacc.Bacc(None, target_bir_lowering=False)

    xt_d = nc.dram_tensor("xt", [NBLK, 128, NCH, 128], F16,
                          kind="ExternalInput")
    wb_d = {bi: nc.dram_tensor(f"wb{bi}", [128, NCH, OC], BF16,
                               kind="ExternalInput") for bi in BF16_PLANES}
    w8_d = {bi: nc.dram_tensor(f"w8{bi}", [128, NPAIR, 2, OC], F8,
                               kind="ExternalInput") for bi in FP8_PLANES}
    c0_d = nc.dram_tensor("c0", [128, NCH, OC], BF16, kind="ExternalInput")
    out_d = nc.dram_tensor("out", [BC, OC], F32, kind="ExternalOutput")

    with tile.TileContext(nc) as tc:
        with (
            tc.tile_pool(name="wpool", bufs=1) as wpool,
            tc.tile_pool(name="const", bufs=1) as cpool,
            tc.tile_pool(name="xs", bufs=SJ + 2) as xspool,
            tc.tile_pool(name="basis", bufs=1) as bpool,
            tc.tile_pool(name="ftmp", bufs=10) as fpool,
            tc.tile_pool(name="outs", bufs=3) as opool,
            tc.tile_pool(name="pbias", bufs=1, space=bass.MemorySpace.PSUM)
                as pbias,
            tc.tile_pool(name="pacc", bufs=2, space=bass.MemorySpace.PSUM)
                as pacc,
            tc.tile_pool(name="pacc8", bufs=4, space=bass.MemorySpace.PSUM)
                as pacc8,
        ):
            onescol_bf = cpool.tile([128, 1], BF16, tag="onescol")
            nc.gpsimd.memset(onescol_bf[:], 1.0)
            ones_bf = cpool.tile([1, 128], BF16, tag="ones")
            nc.gpsimd.memset(ones_bf[:], 1.0)
            bias_bf = cpool.tile([1, OC], BF16, tag="biasbf")
            brow = cpool.tile([1, OC], F32, tag="brow")
            pzs = cpool.tile([128, OC], F32, tag="pzs")
            ones_t = cpool.tile([128, NCH, 128], F16, tag="onest")
            nc.gpsimd.memset(ones_t[:], 1.0)

            def fetch_x(j):
                xs = xspool.tile([128, NCH, 128], F16, tag="xs",
                                 name=f"xs{j}")
                nc.sync.dma_start(xs[:], xt_d[j])
                return xs

            # -- DMA stream: x for first blocks, c0, then W plane-major --
            xs_tiles = {j: fetch_x(j) for j in range(2)}

            c0t = None
            wbt = {}
            w8t = {}

            def stream_w(c0pool):
                # fp8 planes first: smallest, earliest in the basis chain
                nonlocal c0t
                for bi in FP8_PLANES:
                    w8t[bi] = wpool.tile([128, NPAIR, 2, OC], F8, tag="w8",
                                         bufs=len(FP8_PLANES),
                                         name=f"w8_{bi}")
                    nc.sync.dma_start(w8t[bi][:], w8_d[bi][:, :, :, :])
                c0t_ = c0pool.tile([128, NCH, OC], BF16, tag="c0")
                nc.sync.dma_start(c0t_[:], c0_d[:, :, :])
                c0t = c0t_
                for j in range(2, SJ + 2):
                    xs_tiles[j] = fetch_x(j)
                # bf16 planes in chain-readiness order: x first, then T4..T10
                for bi in BF16_PLANES:
                    wbt[bi] = wpool.tile([128, NCH, OC], BF16, tag="wb",
                                         bufs=len(BF16_PLANES),
                                         name=f"wb_{bi}")
                    # two half-plane DMAs: keeps the stream granular enough
                    # for the startup groups to follow it
                    nc.sync.dma_start(wbt[bi][:, 0:NCH // 2, :],
                                      wb_d[bi][:, 0:NCH // 2, :])
                    nc.sync.dma_start(wbt[bi][:, NCH // 2:NCH, :],
                                      wb_d[bi][:, NCH // 2:NCH, :])

            def emit_bias():
                # ones-reduce the c0 plane, broadcast to 128 rows.  Emitted
                # after the startup fp8 groups so the PE isn't ordered to
                # wait on c0's (later) arrival before starting real work.
                pbt = pbias.tile([1, OC], F32, tag="pb", name="pb")
                for c in range(NCH):
                    nc.tensor.matmul(pbt[:], onescol_bf[:], c0t[:, c, :],
                                     start=(c == 0), stop=(c == NCH - 1))
                nc.vector.tensor_copy(brow[:], pbt[:])
                nc.vector.tensor_copy(bias_bf[:], brow[:])
                pz = pbias.tile([128, OC], F32, tag="pz", name="pz")
                nc.tensor.matmul(pz[:], ones_bf[:], bias_bf[:],
                                 start=True, stop=True)
                nc.vector.tensor_copy(pzs[:], pz[:])

            # -- basis chain.  qw>1 slices every op into qw chunk-groups,
            # emitted quarter-major: slashes the serial-spine latency for
            # the startup blocks (matmul groups consume pair p as soon as
            # its slice is written — the Tile framework tracks subtile
            # deps), at the cost of qw x the instruction count.
            # All chain intermediates are fp16: DVE runs 2-byte all-SBUF
            # TensorTensor at 4x (327ns/block-op) and STT at 2x (594ns),
            # vs 1067ns for fp32.  Squares are DVE multiplies; "2x-1" is an
            # STT against a ones tile (dual-immediate tensor_scalar gets no
            # fast mode).  fp8 casts (1-byte out breaks DVE fast modes) go
            # to ACT; plain copies to Pool.  fp16 rounding (~5e-4 on [-1,1])
            # is negligible next to the bf16/fp8 operand casts.
            def basis_chain(j, xs, qw=1):
                bas = {}
                for bi in range(NKB):
                    dt = F8 if bi in FP8_PLANES else BF16
                    bas[bi] = bpool.tile([128, NCH, 128], dt, tag=f"bas{bi}",
                                         bufs=2, name=f"bas{j}_{bi}")
                t_f = fpool.tile([128, NCH, 128], F16, tag="ftmp",
                                 name=f"t{j}")
                tf = {1: t_f}
                for m in (2, 3, 4, 5):
                    tf[m] = fpool.tile([128, NCH, 128], F16, tag="ftmp",
                                       name=f"tf{j}_{m}")
                tp = {m: fpool.tile([128, NCH, 128], F16, tag="ftmp",
                                    name=f"tp{j}_{m}")
                      for m in (2, 3, 4, 5, 6, 7, 8, 9, 10)}

                QC = NCH // qw
                for q in range(qw):
                    s = (slice(None), slice(q * QC, (q + 1) * QC),
                         slice(None))

                    def cast(bi, src):
                        if bi in FP8_PLANES:
                            nc.scalar.activation(bas[bi][s], src[s],
                                                 AF.Copy, scale=SA)
                        else:
                            nc.gpsimd.tensor_copy(bas[bi][s], src[s])

                    def step(a, b, sub_t, dst_f, dst_bi):
                        # T = 2*a*b - sub_t  (sub_t: ones tile or t)
                        tmp = tp[dst_bi]
                        nc.vector.tensor_tensor(tmp[s], a[s], b[s], OP.mult)
                        dst = bas[dst_bi] if dst_f is None else dst_f
                        nc.vector.scalar_tensor_tensor(
                            dst[s], tmp[s], 2.0, sub_t[s],
                            OP.mult, OP.subtract)
                        if dst_f is not None:
                            cast(dst_bi, dst_f)

                    nc.scalar.activation(t_f[s], xs[s], AF.Tanh)
                    nc.scalar.activation(bas[0][s], t_f[s], AF.Copy,
                                         scale=SA)                 # T1 fp8
                    nc.gpsimd.tensor_copy(bas[1][s], xs[s])        # x bf16
                    step(t_f, t_f, ones_t, tf[2], 2)      # T2 = 2t^2-1
                    step(t_f, tf[2], t_f, tf[3], 3)       # T3
                    step(tf[2], tf[2], ones_t, tf[4], 4)  # T4
                    step(tf[2], tf[3], t_f, tf[5], 5)     # T5
                    step(tf[3], tf[3], ones_t, None, 6)   # T6
                    step(tf[3], tf[4], t_f, None, 7)      # T7
                    step(tf[4], tf[4], ones_t, None, 8)   # T8
                    step(tf[4], tf[5], t_f, None, 9)      # T9
                    step(tf[5], tf[5], ones_t, None, 10)  # T10
                return bas

            # matmul emit order within groups: readiness order of the chain
            FP8_ORDER = sorted(FP8_PLANES)                   # T1, T2, T3(,T4)
            BF16_ORDER = [1] + [bi for bi in BF16_PLANES if bi != 1]

            def fp8_group(bas, h, psum_tile):
                """One DoubleRow group: batch half h -> psum [64, OC]."""
                n = len(FP8_ORDER) * NPAIR * 2
                i = 0
                for bi in FP8_ORDER:
                    for p in range(NPAIR):
                        for oh in (0, 1):
                            nc.tensor.matmul(
                                psum_tile[:, oh * HALF:(oh + 1) * HALF],
                                bas[bi][:, 2 * p:2 * p + 2,
                                        h * 64:(h + 1) * 64],
                                w8t[bi][:, p, :, oh * HALF:(oh + 1) * HALF],
                                start=(i == 0), stop=(i == n - 1),
                                perf_mode=PM.DoubleRow)
                            i += 1

            def bf16_group(bas, psum_tile):
                n = len(BF16_ORDER) * NCH
                i = 0
                for bi in BF16_ORDER:
                    for c in range(NCH):
                        nc.tensor.matmul(psum_tile[:], bas[bi][:, c, :],
                                         wbt[bi][:, c, :],
                                         start=(i == 0), stop=(i == n - 1))
                        i += 1

            # fp8 half-block results live at psum partitions 0-63.  The PE
            # cannot write psum at a partition offset and DVE operands must
            # share partition offsets, so the upper batch half is moved with
            # a SBUF->SBUF DMA (partition-agnostic) into rows 64-127 of a
            # [128, OC] staging tile before the full-width combine.
            def drain_fp8(j, p8lo_ap, p8hi_ap, spool):
                """scale fp8 psum halves into a [128, OC] bf16 staging tile
                (upper half via the SBUF->SBUF bounce)."""
                s8 = spool.tile([128, OC], BF16, tag="s8", name=f"s8_{j}")
                shi = spool.tile([64, OC], BF16, tag="shi", name=f"shi_{j}")
                nc.vector.tensor_scalar(s8[0:64, :], p8lo_ap, SINV, None,
                                        OP.mult)
                nc.vector.tensor_scalar(shi[:], p8hi_ap, SINV, None, OP.mult)
                nc.sync.dma_start(s8[64:128, :], shi[:])
                return s8

            def finish(j, s8, accbf_ap):
                ob = opool.tile([128, OC], F32, tag="ob", name=f"ob{j}")
                nc.vector.tensor_tensor(ob[:], s8[:], accbf_ap, OP.add)
                nc.vector.tensor_tensor(ob[:], ob[:], pzs[:], OP.add)
                nc.sync.dma_start(out_d[j * 128:(j + 1) * 128, :], ob[:])

            # -- startup: per-plane groups following the W stream.
            # Accumulators come from persistent pools: scoped pools would
            # emit close-barriers that serialize the steady pipeline behind
            # the startup drains.  c0 gets its own scoped pool (its last
            # reader is the bias reduction, early enough to be harmless).
            c0scope = tc.tile_pool(name="c0pool", bufs=1)
            c0pool = c0scope.__enter__()
            stream_w(c0pool)
            bas_s = {}
            sacc = {}
            s8acc = {}
            for j in range(SJ):
                bas_s[j] = basis_chain(j, xs_tiles[j], qw=2)
                sacc[j] = opool.tile([128, OC], F32, tag="sacc", bufs=SJ,
                                     name=f"sacc{j}")
                nc.gpsimd.memset(sacc[j][:], 0.0)
                for h in (0, 1):
                    s8acc[(j, h)] = opool.tile([64, OC], BF16, tag="sacc8",
                                               bufs=2 * SJ,
                                               name=f"s8a{j}_{h}")
                    nc.gpsimd.memset(s8acc[(j, h)][:], 0.0)

            for bi in FP8_ORDER:
                for j in range(SJ):
                    for h in (0, 1):
                        p8 = pacc8.tile([64, OC], F32, tag="acc8",
                                        name=f"sp8_{j}_{bi}_{h}")
                        m = NPAIR * 2
                        i = 0
                        for p in range(NPAIR):
                            for oh in (0, 1):
                                nc.tensor.matmul(
                                    p8[:, oh * HALF:(oh + 1) * HALF],
                                    bas_s[j][bi][:, 2 * p:2 * p + 2,
                                                 h * 64:(h + 1) * 64],
                                    w8t[bi][:, p, :,
                                            oh * HALF:(oh + 1) * HALF],
                                    start=(i == 0), stop=(i == m - 1),
                                    perf_mode=PM.DoubleRow)
                                i += 1
                        nc.vector.tensor_tensor(
                            s8acc[(j, h)][:], s8acc[(j, h)][:], p8[:],
                            OP.add)
            emit_bias()
            c0scope.__exit__(None, None, None)
            for bi in BF16_ORDER:
                for j in range(SJ):
                    pb = pacc.tile([128, OC], F32, tag="acc",
                                   name=f"spb_{j}_{bi}")
                    for c in range(NCH):
                        nc.tensor.matmul(pb[:], bas_s[j][bi][:, c, :],
                                         wbt[bi][:, c, :],
                                         start=(c == 0),
                                         stop=(c == NCH - 1))
                    nc.vector.tensor_tensor(sacc[j][:], sacc[j][:],
                                            pb[:], OP.add)
            for j in range(SJ):
                s8 = drain_fp8(j, s8acc[(j, 0)][:], s8acc[(j, 1)][:],
                               opool)
                finish(j, s8, sacc[j][:])

            # -- steady state --------------------------------------------
            def matmuls(j, bas):
                acc8 = {h: pacc8.tile([64, OC], F32, tag="acc8",
                                      name=f"a8_{j}_{h}")
                        for h in (0, 1)}
                accbf = pacc.tile([128, OC], F32, tag="acc",
                                  name=f"ab_{j}")
                for h in (0, 1):
                    fp8_group(bas, h, acc8[h])
                # drain+bounce the fp8 halves while the bf16 group runs
                s8 = drain_fp8(j, acc8[0][:], acc8[1][:], opool)
                bf16_group(bas, accbf)
                finish(j, s8, accbf[:])

            bas_prev = basis_chain(SJ, xs_tiles[SJ])
            for j in range(SJ + 1, NBLK):
                xs_j = fetch_x(j)
                matmuls(j - 1, bas_prev)
                bas_prev = basis_chain(j, xs_j)
            matmuls(NBLK - 1, bas_prev)

    nc.compile()
    return nc


def _prep_core_inputs(x, coeffs, base_weight):
    """Host-side shard + layout prep.  Returns in_maps for the 8 cores."""
    bf16 = ml_dtypes.bfloat16
    f8 = ml_dtypes.float8_e4m3

    in_maps = []
    # per out-shard W prep (shared across the 4 batch shards)
    wsets = []
    for o in range(MO):
        osl = slice(o * OC, (o + 1) * OC)
        m = {}
        for bi in BF16_PLANES:
            # plane 0 is T1 -> coeffs n=1; plane 1 is x -> base_weight
            wn = (base_weight[:, osl] if bi == 1
                  else coeffs[:, osl, 1 if bi == 0 else bi])
            # [IN, OC] -> [128, NCH, OC] (partition-major)
            m[f"wb{bi}"] = np.ascontiguousarray(
                wn.reshape(NCH, 128, OC).transpose(1, 0, 2).astype(bf16))
        for bi in FP8_PLANES:
            wn = coeffs[:, osl, 1 if bi == 0 else bi]
            q = (wn * SW).astype(f8)
            # [IN, OC] -> [128, NPAIR, 2, OC]: slab i of pair p is chunk 2p+i
            m[f"w8{bi}"] = np.ascontiguousarray(
                q.reshape(NPAIR, 2, 128, OC).transpose(2, 0, 1, 3))
        m["c0"] = np.ascontiguousarray(
            coeffs[:, osl, 0].reshape(NCH, 128, OC).transpose(1, 0, 2)
            .astype(bf16))
        wsets.append(m)

    for core in range(8):
        b_idx, o_idx = divmod(core, MO)
        xs = x[b_idx * BC:(b_idx + 1) * BC, :]
        # [BC, IN] -> [NBLK, 128part(i), NCH, 128(batch)]
        xt = np.ascontiguousarray(
            xs.reshape(NBLK, 128, NCH, 128).transpose(0, 3, 2, 1)
            .astype(np.float16))
        d = {"xt": xt}
        d.update(wsets[o_idx])
        in_maps.append(d)
    return in_maps


def kernel(x, coeffs, base_weight):
    global LAST_RESULTS
    assert x.shape == (B, IN) and coeffs.shape == (IN, OUT, DEG + 1)
    assert base_weight.shape == (IN, OUT)

    if "nc" not in _CACHE:
        _CACHE["nc"] = _build_nc()
    nc = _CACHE["nc"]

    x = np.ascontiguousarray(x, dtype=np.float32)
    coeffs = np.ascontiguousarray(coeffs, dtype=np.float32)
    base_weight = np.ascontiguousarray(base_weight, dtype=np.float32)
    assert np.abs(coeffs[:, :, list(FP8_PLANES)]).max() * SW < 240.0

    in_maps = _prep_core_inputs(x, coeffs, base_weight)
    res = run_bass_kernel_spmd(nc, in_maps, core_ids=list(range(8)))
    LAST_RESULTS = res

    out = np.empty((B, OUT), dtype=np.float32)
    for core in range(8):
        b_idx, o_idx = divmod(core, MO)
        out[b_idx * BC:(b_idx + 1) * BC, o_idx * OC:(o_idx + 1) * OC] = \
            res.results[core]["out"]
    return out
